# revision 3
# baseline (speedup 1.0000x reference)
"""Trainium2 Bass kernel for 2-layer GAT (nn_GAT_22634477650567), v4.

8 NeuronCores, tensor-parallel over H=8 heads (one head per core).
T-major layout ([feature, node]).

Per layer:
  - scores pp^T[k,q] = exp(lrelu(src_q + dst_k) - C) in bf16, C=4 shift.
  - score-gen split: 17 DVE chunks (custom MAXPROD, 1 op) and 15 ACT
    chunks (Prelu then in-place Exp), chosen to balance engine time.
  - mask: multiplicative {1,0}; stored fp8e4m3 in HBM, cast to bf16 by a
    plain SWDGE DMA (one DMA per pair of chunks), then applied in-place by
    tensor_mul: 19 chunks on DVE (2x mode) and 13 on GPSIMD (third lane).
  - apply: bf16 matmuls whb[128,c,33] x pp[128,512] accumulated in
    PSUM acc[33, 4096]; 33rd weight column of ones gives the denominator.
  - engine streams software-pipelined at pair granularity.
  - h bf16, residuals in place; AllGather bf16.

(The earlier DMA-accum masking designs are dead: NEFF verifier only allows
cce add, and accum DMAs crash the accelerator at runtime.)
"""

import os
import numpy as np
import ml_dtypes

import concourse.bass as bass
import concourse.mybir as mybir
import concourse.tile as tile
from concourse import bacc
from concourse.bass_utils import run_bass_kernel_spmd

import concourse.dve_ops as dve_ops
from concourse.dve_spec import (
    Src0,
    Src1,
    C0,
    C1,
    maxx,
    lower as dve_lower,
    Spec as DveSpec,
)
from concourse.dve_uop import DveOpSpec


def _register_maxprod():
    name = "MAXPROD_ANT"
    for op in dve_ops.OPS:
        if op.name == name:
            return op
    spec = DveSpec(
        body=maxx(Src0 * C0, Src1 * C1),
        reference=lambda in0, in1, s0, s1, imm2: np.maximum(in0 * s0, in1 * s1).astype(
            np.float32
        ),
    )
    opcode = dve_ops._CUSTOM_DVE_ROW_BASE + len(dve_ops.OPS)
    shas = {}
    for ver in ("v3", "v4"):
        s = DveOpSpec(
            name=name, opcode=opcode, uops=dve_lower(spec, ver=ver), rd1_en=True
        )
        shas[ver] = s.sha(ver)
    op = dve_ops.DveOp(name, spec, subdim=False, uops_sha=shas)
    dve_ops.OPS.append(op)
    dve_ops.CUSTOM_DVE_SPECS[name] = spec
    dve_ops._SUB_OPCODE_FOR_NAME[name] = opcode
    return op


MAXPROD = _register_maxprod()

F32 = mybir.dt.float32
BF16 = mybir.dt.bfloat16
FP8 = mybir.dt.float8e4
AF = mybir.ActivationFunctionType
ALU = mybir.AluOpType

N = 4096          # nodes
D = 256           # input features
O = 32            # per-head output features
P = 128           # partitions
NCH = N // P      # 32 k-chunks
NPAIR = NCH // 2
NB = N // 512     # 8 psum bank columns
NCORE = 8
LRELU = 0.2
CSHIFT = 4.0      # softmax shift: pp = exp(lrelu(z) - CSHIFT)

# score-gen engine per chunk: 15 ACT chunks spread among 32
_ALL_DVE = bool(int(os.environ.get("GAT_ALL_DVE", "0")))
ACT_SET = (
    frozenset()
    if _ALL_DVE
    else frozenset([1, 3, 5, 7, 9, 11, 13, 15, 17, 19, 21, 23, 25, 27, 29, 31])
)
# mask-mul engine per chunk: 15 on POOL, 17 on DVE
POOL_MASK = frozenset([0, 2, 4, 6, 8, 10, 12, 14, 16, 18, 20, 22, 24, 26, 28, 30])


def _gat_layer(nc, tc, pools, layer, xt_tiles, w_dram, a_dram, mask_dram):
    """One GAT head layer. Returns normalized head output [32, 4096] bf16."""
    sb = pools["sb"]
    big = pools["big"]
    L = layer

    wsb = sb.tile([P, 2 * O], BF16, name=f"wsb{L}", tag="wsb")
    nc.sync.dma_start(wsb[:, 0:O], w_dram[0:P, :])
    nc.sync.dma_start(wsb[:, O:2 * O], w_dram[P:D, :])
    asb = sb.tile([O, 2], BF16, name=f"asb{L}", tag="asb")
    nc.sync.dma_start(asb[:], a_dram[:])
    onesb = sb.tile([1, P], BF16, name=f"onesb{L}", tag="onesb")
    nc.vector.memset(onesb[:], 1.0)
    ones32 = sb.tile([1, O], BF16, name=f"ones32{L}", tag="ones32")
    nc.vector.memset(ones32[:], 1.0)
    dcol = sb.tile([P, 4 * NCH], F32, name=f"dcol{L}", tag="dcol")
    DC_RAW, DC_E, DC_E2 = 0, NCH, 2 * NCH
    cst = sb.tile([P, 2], F32, name=f"cst{L}", tag="cst")
    nc.vector.memset(cst[:, 0:1], -CSHIFT / 2)
    nc.vector.memset(cst[:, 1:2], -CSHIFT)

    whT = sb.tile([O, N], BF16, name=f"whT{L}", tag="whT")
    whb = sb.tile([P, NCH, O + 1], BF16, name=f"whb{L}", tag="whb")
    nc.vector.memset(whb[:], 1.0)

    sbc = big.tile([P, N], BF16, name=f"sbc{L}", tag="sbc")
    ubc = big.tile([P, N], BF16, name=f"ubc{L}", tag="ubc")
    abc = big.tile([P, N], BF16, name=f"abc{L}", tag="abc")

    with (
        tc.tile_pool(name=f"sps{L}", bufs=4, space="PSUM") as sps,
        tc.tile_pool(name=f"spt{L}", bufs=1, space="PSUM") as spt,
    ):
        # Wh n-major chunks -> whb bf16 (col 32 stays ones)
        for c in range(NCH):
            pw = sps.tile([P, O], F32, name=f"pw{L}_{c}", tag="ps")
            for dc in range(2):
                nc.tensor.matmul(
                    pw[:],
                    xt_tiles[dc][:, c * P:(c + 1) * P],
                    wsb[:, dc * O:(dc + 1) * O],
                    start=(dc == 0),
                    stop=(dc == 1),
                )
            nc.any.tensor_copy(whb[:, c, 0:O], pw[:])

        # WhT [o, n] bf16
        for h in range(2):
            pt = spt.tile([O, 2048], F32, name=f"pt{L}_{h}", tag="pt")
            for g in range(4):
                for dc in range(2):
                    nc.tensor.matmul(
                        pt[:, g * 512:(g + 1) * 512],
                        wsb[:, dc * O:(dc + 1) * O],
                        xt_tiles[dc][:, h * 2048 + g * 512: h * 2048 + (g + 1) * 512],
                        start=(dc == 0),
                        stop=(dc == 1),
                    )
            nc.any.tensor_copy(whT[:, h * 2048:(h + 1) * 2048], pt[:])

        # src row -> sbc row 0
        for g in range(NB):
            pr = sps.tile([1, 512], F32, name=f"pr{L}_{g}", tag="ps")
            nc.tensor.matmul(
                pr[:], asb[:, 0:1],
                whT[:, g * 512:(g + 1) * 512], start=True, stop=True,
            )
            nc.any.tensor_copy(sbc[0:1, g * 512:(g + 1) * 512], pr[:])

        # dst col [128, 32] + exps
        dps = sps.tile([P, NCH], F32, name=f"dps{L}", tag="ps")
        for c in range(NCH):
            nc.tensor.matmul(
                dps[:, c:c + 1], whT[:, c * P:(c + 1) * P],
                asb[:, 1:2], start=True, stop=True,
            )
        nc.vector.tensor_copy(dcol[:, DC_RAW:DC_RAW + NCH], dps[:])
        nc.scalar.activation(dcol[:, DC_E:DC_E + NCH],
                             dcol[:, DC_RAW:DC_RAW + NCH], AF.Exp,
                             bias=cst[:, 0:1])
        nc.scalar.activation(dcol[:, DC_E2:DC_E2 + NCH],
                             dcol[:, DC_RAW:DC_RAW + NCH], AF.Exp,
                             scale=LRELU, bias=cst[:, 0:1])

        # src broadcast
        for g in range(NB):
            pb = sps.tile([P, 512], F32, name=f"pb{L}_{g}", tag="ps")
            nc.tensor.matmul(
                pb[:], onesb[:],
                sbc[0:1, g * 512:(g + 1) * 512], start=True, stop=True,
            )
            nc.any.tensor_copy(sbc[:, g * 512:(g + 1) * 512], pb[:])

    nc.scalar.activation(ubc[:], sbc[:], AF.Exp, bias=cst[:, 0:1])
    nc.scalar.activation(abc[:], sbc[:], AF.Exp, scale=LRELU, bias=cst[:, 0:1])

    num = big.tile([O, N], BF16, name=f"num{L}", tag="num")
    drow = sb.tile([1, N], BF16, name=f"drow{L}", tag="drow")

    # ---------------- hot loop ----------------
    with tc.tile_pool(name=f"aps{L}", bufs=1, space="PSUM") as aps:
        acc = aps.tile([O + 1, N], F32, name=f"acc{L}")
        nmm_bank = [0] * NB

        def emit_apply(c, sp, slot):
            for g in range(NB):
                i = nmm_bank[g]
                nc.tensor.matmul(
                    acc[:, g * 512:(g + 1) * 512],
                    whb[:, c, :],
                    sp[:, slot, g * 512:(g + 1) * 512],
                    start=(i == 0),
                    stop=(i == NCH - 1),
                )
                nmm_bank[g] += 1

        def prework(c, sp, slot):
            if c in ACT_SET:
                nc.scalar.activation(sp[:, slot, :], sbc[:], AF.Prelu,
                                     bias=dcol[:, DC_RAW + c:DC_RAW + c + 1],
                                     alpha=LRELU)
                nc.scalar.activation(sp[:, slot, :], sp[:, slot, :], AF.Exp,
                                     bias=cst[:, 1:2])
            else:
                nc.vector._custom_dve(
                    MAXPROD, out=sp[:, slot, :], in0=ubc[:], in1=abc[:],
                    s0=dcol[:, DC_E + c:DC_E + c + 1],
                    s1=dcol[:, DC_E2 + c:DC_E2 + c + 1],
                )

        def finish(st):
            p, sp, mk = st
            for slot in range(2):
                c = 2 * p + slot
                eng = nc.gpsimd if c in POOL_MASK else nc.vector
                eng.tensor_mul(sp[:, slot, :], sp[:, slot, :], mk[:, slot, :])
                emit_apply(c, sp, slot)

        pend = []
        for p in range(NPAIR):
            # prefetch mask pair (fp8 -> bf16 cast during SWDGE DMA)
            mk = pools["mk"].tile([P, 2, N], BF16, name=f"mk{L}_{p}", tag="mk")
            nc.sync.dma_start(
                mk[:],
                mask_dram[2 * p * P:(2 * p + 2) * P, :].rearrange(
                    "(i p) q -> p i q", i=2),
            )
            sp = pools["sp"].tile([P, 2, N], BF16, name=f"sp{L}_{p}", tag="sp")
            prework(2 * p, sp, 0)
            prework(2 * p + 1, sp, 1)
            pend.append((p, sp, mk))
            if len(pend) > 1:
                finish(pend.pop(0))
        while pend:
            finish(pend.pop(0))
        assert all(n == NCH for n in nmm_bank)

        nc.scalar.copy(drow[:], acc[O:O + 1, :])
        nc.scalar.copy(num[:], acc[0:O, :])

    # ---- normalize ----
    den = sb.tile([P, O], BF16, name=f"den{L}", tag="den")
    denr = sb.tile([P, O], F32, name=f"denr{L}", tag="denr")
    denb = sb.tile([P, O], BF16, name=f"denb{L}", tag="denb")
    drb = sb.tile([1, N], BF16, name=f"drb{L}", tag="drb")
    nc.sync.dma_start(den[:], drow[:])
    nc.vector.tensor_copy(denr[:], den[:])
    nc.vector.reciprocal(denr[:], denr[:])
    nc.vector.tensor_copy(denb[:], denr[:])
    nc.sync.dma_start(drb[:], denb[:])

    on = big.tile([O, N], BF16, name=f"on{L}", tag="on")
    with tc.tile_pool(name=f"rps{L}", bufs=1, space="PSUM") as rps:
        rb = rps.tile([O, N], F32, name=f"rb{L}")
        for g in range(NB):
            nc.tensor.matmul(
                rb[:, g * 512:(g + 1) * 512],
                ones32[:],
                drb[:, g * 512:(g + 1) * 512], start=True, stop=True,
            )
        nc.vector.tensor_mul(on[:], num[:], rb[:])
    return on


def _elu_residual(nc, pools, name, ct, res, dst, rows=P, eng=None):
    """dst[0:rows] = elu(ct[0:rows]) + res[0:rows]. eng picks the ALU lane
    (nc.vector default; nc.gpsimd lets two residual halves run in parallel)."""
    if eng is None:
        eng = nc.vector
    if eng is nc.gpsimd:
        # borrow a mask-pool pair tile so both residual halves can run in
        # parallel (the yy pool only holds one half's scratch at a time)
        pair = pools["mk"].tile([P, 2, N], BF16, name=f"sc{name}", tag="mk")
        t1, t2 = pair[:, 0], pair[:, 1]
    else:
        t1 = pools["yy"].tile([P, N], BF16, name=f"t1{name}", tag="yy")
        t2 = pools["yy"].tile([P, N], BF16, name=f"t2{name}", tag="yy")
    r = rows
    eng.tensor_scalar_min(t1[0:r, :], ct[0:r, :], 0.0)
    nc.scalar.activation(t2[0:r, :], t1[0:r, :], AF.Exp)
    eng.tensor_scalar(t1[0:r, :], ct[0:r, :], 0.0, -1.0, ALU.max, ALU.add)
    eng.tensor_add(t2[0:r, :], t1[0:r, :], t2[0:r, :])
    eng.tensor_add(dst[0:r, :], t2[0:r, :], res[0:r, :])


def build_kernel(repeat=1, no_collective=False):
    nc = bacc.Bacc("TRN2", target_bir_lowering=False, debug=False,
                   num_devices=NCORE)

    xT_d = nc.dram_tensor("xTb", [D, N], BF16, kind="ExternalInput")
    xTown_d = nc.dram_tensor("xTownb", [O, N], BF16, kind="ExternalInput")
    w1_d = nc.dram_tensor("w1b", [D, O], BF16, kind="ExternalInput")
    w2_d = nc.dram_tensor("w2b", [D, O], BF16, kind="ExternalInput")
    a1_d = nc.dram_tensor("a1", [O, 2], BF16, kind="ExternalInput")
    a2_d = nc.dram_tensor("a2", [O, 2], BF16, kind="ExternalInput")
    mask_d = nc.dram_tensor("maskmul", [N, N], BF16, kind="ExternalInput")
    outT_d = nc.dram_tensor("outT", [O, N], F32, kind="ExternalOutput")

    with tile.TileContext(nc) as tc:
        with (
            tc.tile_pool(name="sb", bufs=1) as sb,
            tc.tile_pool(name="big", bufs=1) as big,
            tc.tile_pool(name="sp", bufs=2) as sp_pool,
            tc.tile_pool(name="mk", bufs=2) as mk_pool,
            tc.tile_pool(name="yy", bufs=2) as yy_pool,
            tc.tile_pool(name="dram", bufs=1, space="DRAM") as dram,
        ):
            pools = dict(sb=sb, big=big, sp=sp_pool, mk=mk_pool, yy=yy_pool)

            for rep in range(repeat):
                xt0 = big.tile([P, N], BF16, name=f"xt0_{rep}", tag="hx0")
                nc.sync.dma_start(xt0[:], xT_d[0:P, :])
                xt1 = big.tile([P, N], BF16, name=f"xt1_{rep}", tag="hx1")
                nc.sync.dma_start(xt1[:], xT_d[P:D, :])

                o1n = _gat_layer(nc, tc, pools, 10 * rep + 1, (xt0, xt1),
                                 w1_d, a1_d, mask_d)

                gin = dram.tile([O, N], BF16, name=f"gin{rep}")
                nc.sync.dma_start(gin[:], o1n[:])
                catT = dram.tile([D, N], BF16, name=f"catT{rep}",
                                 addr_space="Local" if no_collective else "Shared")
                if no_collective:
                    for jj in range(NCORE):
                        nc.sync.dma_start(catT[jj * O:(jj + 1) * O, :], gin[:])
                else:
                    nc.gpsimd.collective_compute(
                        "AllGather", ALU.bypass,
                        replica_groups=[list(range(NCORE))],
                        ins=[gin.opt()], outs=[catT.opt()],
                    )

                xown = sb.tile([O, N], BF16, name=f"xown{rep}", tag="xown")
                nc.sync.dma_start(xown[:], xTown_d[:])
                hown = sb.tile([O, N], BF16, name=f"hown{rep}", tag="hown")
                _elu_residual(nc, pools, f"ho{rep}", o1n, xown, hown, rows=O)

                for half, xt in ((0, xt0), (1, xt1)):
                    ct = big.tile([P, N], BF16, name=f"ct{half}_{rep}", tag="abc")
                    nc.sync.dma_start(ct[:], catT[half * P:(half + 1) * P, :])
                    _elu_residual(nc, pools, f"h{half}_{rep}", ct, xt, xt)

                o2n = _gat_layer(nc, tc, pools, 10 * rep + 2, (xt0, xt1),
                                 w2_d, a2_d, mask_d)

                outsb = sb.tile([O, N], BF16, name=f"outsb{rep}", tag="outsb")
                _elu_residual(nc, pools, f"f{rep}", o2n, hown, outsb, rows=O)
                nc.gpsimd.dma_start(outT_d[:], outsb[:])

    nc.compile()
    return nc


_NC_CACHE = None


def _get_nc():
    global _NC_CACHE
    if _NC_CACHE is None:
        _NC_CACHE = build_kernel()
    return _NC_CACHE


def kernel(x, adj_mat, W1, a1, W2, a2, _trace=False, _tmpdir=None):
    x = np.asarray(x, dtype=np.float32)
    adj = np.asarray(adj_mat)
    W1 = np.asarray(W1, dtype=np.float32)
    a1 = np.asarray(a1, dtype=np.float32)
    W2 = np.asarray(W2, dtype=np.float32)
    a2 = np.asarray(a2, dtype=np.float32)

    xTb = np.ascontiguousarray(x.T).astype(ml_dtypes.bfloat16)
    maskmul = (adj.T > 0).astype(ml_dtypes.bfloat16)

    nc = _get_nc()
    in_maps = []
    for j in range(NCORE):
        in_maps.append(
            dict(
                xTb=xTb,
                xTownb=np.ascontiguousarray(xTb[j * O:(j + 1) * O]),
                w1b=np.ascontiguousarray(W1[j]).astype(ml_dtypes.bfloat16),
                w2b=np.ascontiguousarray(W2[j]).astype(ml_dtypes.bfloat16),
                a1=np.ascontiguousarray(np.stack([a1[j, :O], a1[j, O:]], axis=1)).astype(ml_dtypes.bfloat16),
                a2=np.ascontiguousarray(np.stack([a2[j, :O], a2[j, O:]], axis=1)).astype(ml_dtypes.bfloat16),
                maskmul=maskmul,
            )
        )
    kw = {}
    if _trace:
        kw = dict(trace=True, tmpdir=_tmpdir)
    res = run_bass_kernel_spmd(nc, in_maps, list(range(NCORE)), **kw)
    out = np.empty((N, NCORE * O), dtype=np.float32)
    for j in range(NCORE):
        out[:, j * O:(j + 1) * O] = res.results[j]["outT"].T
    if _trace:
        return out, res
    return out
